# revision 15
# baseline (speedup 1.0000x reference)
"""Causal self-attention (B=2, T=2048, C=1024, H=16) on 8 Trainium2 cores.

Sharding: data-parallel over batch (2) x tensor-parallel over heads (4 groups
of 4 heads). Core c handles batch b = c//4, head group g = c%4 (heads 4g..4g+3).
Each core computes its qkv column slice, full causal TxT attention for its 4
heads, and a partial row-parallel projection. Host sums the 4 partial proj
outputs per batch and adds b_proj.

Device kernel layout notes (v3):
- all matmul operands are fp16: on TRN2 fp32r runs fp32_mode=HIGH (2 PE
  passes -> 2 cycles/row, doubled LDWEIGHTS, and the row-bank conflict
  serializes the two K=64 head-quadrant score matmuls). fp16 is 1 cycle/row,
  enables FWL weight loads, and the hi=0/hi=1 score matmuls (stationary rows
  0-63 / 64-127) genuinely overlap (measured dstart ~4ns). PSUM stays fp32.
- feature-major ("transposed") layouts throughout: qT/kT [d, t] so PE
  contraction dims line up with no on-device transposes
- host pre-packs every DRAM tensor so each DMA moves 2-8KB contiguous
  per-partition lines (small strided lines measured ~85GB/s/queue; packed
  ~200+GB/s), and the first-needed tensors (wq col group 0, x t-quarter 0)
  are triggered first; all small constants ship as one [128, 516] f16 blob
- softmax without max-subtraction (logits ~N(0,1), exp fits fp16); exp runs
  on the ACT engine - the second binding resource (~70us) after the PE
  (~100us); the AV stationary operand embeds an all-ones column so each AV
  matmul also emits the softmax denominator row for free
- denominator rows are partition-broadcast with a single K=128 matmul against
  a constant selector matrix (col j<64 reads row 64 = den0, col j>=64 reads
  row 0 = den1) over a pre-zeroed staging row-pair; reciprocal via the
  custom-DVE reciprocal_approx_fast (~5x faster than exact reciprocal)
- causal masking: upper-triangle j-chunks skipped; diagonal chunks narrow the
  score/exp/AV column range to [o*128, 512) and one [128,128] triangular
  multiplicative mask handles the partial strip
- scheduling: the score matmul for chunk jc+1 issues before the AV for jc so
  the in-order PE stream never waits on ACT; "filler" PE work (projection of
  finished i-chunks + deferred phase-1 work: the last t-quarter of qT/kT and
  v chunks 12-15) is woven in at a fixed cadence to keep the PE at the ACT
  rate. Dense PE occupancy also parks the HAM clock gate at 2.4 GHz (any
  ~3.4us idle window re-throttles the PE to 1.2 GHz).
- normalization lags one block: den-row copies (DVE) issue right after the
  block's last AV, the broadcast-matmul + reciprocal + yT scale are emitted
  inside the NEXT block so the PE never stalls on DVE latency
- PSUM budget (8 banks): "s" tag [128,2,512] bufs=2 (scores, proj, den
  broadcast, and phase-1 qkv groups all share it) + av0/av1 bufs=2 each
- psum->sbuf copies for the projection ride the GpSimd engine (DVE is the
  #3 resource); output tiles stream out per-tile as fp16 on the sync queue
"""

import os
import sys

sys.path.insert(0, "/opt/trn_rl_repo")

import numpy as np

P = 128
T = 2048
C = 1024
D = 64
HPC = 4          # heads per core
HD = HPC * D     # 256 qkv columns per core
CC = C // P      # 8 contraction chunks
TC = T // P      # 16 t-chunks of 128
IC = T // 512    # 4 i-chunks of 512

# const blob column offsets
OFF_TRI = 0
OFF_SEL = 128
OFF_BQ = 256
OFF_BK = 258
OFF_BV = 260
CSTW = 516

_NC = None
LAST_RESULTS = None


def _build_nc():
    import concourse.mybir as mybir
    import concourse.tile as tile
    from concourse import bacc
    from contextlib import ExitStack

    dt = mybir.dt
    f32 = dt.float32
    f16 = dt.float16
    ALU = mybir.AluOpType
    ACTF = mybir.ActivationFunctionType

    nc = bacc.Bacc(
        "TRN2",
        target_bir_lowering=False,
        debug=False,
        enable_asserts=False,
        num_devices=8,
    )

    # host-packed layouts: contiguous per-partition lines per transfer
    xq = nc.dram_tensor("xq", [P, 4, CC, 512], f16, kind="ExternalInput").ap()
    wq2 = nc.dram_tensor("wq2", [P, 2, CC, P], f16, kind="ExternalInput").ap()
    wk2 = nc.dram_tensor("wk2", [P, 2, CC, P], f16, kind="ExternalInput").ap()
    wv2 = nc.dram_tensor("wv2", [P, CC, HD], f16, kind="ExternalInput").ap()
    wp2 = nc.dram_tensor("wp2", [P, 2, C], f16, kind="ExternalInput").ap()
    cst = nc.dram_tensor("cst", [P, CSTW], f16, kind="ExternalInput").ap()
    out = nc.dram_tensor("out", [T, C], f16, kind="ExternalOutput").ap()

    with tile.TileContext(nc) as tc, ExitStack() as ctx:
        persist = ctx.enter_context(tc.tile_pool(name="persist", bufs=1))
        qT_sb = persist.tile([P, 2, T], f16, name="qT")    # [d%128, dchunk, t]
        kT_sb = persist.tile([P, 2, T], f16, name="kT")
        v_sb = persist.tile([P, TC, 2, 2, P], f16, name="v")  # [t%128, tchunk, hpair, hi, 128-padded d]
        yT_sb = persist.tile([P, 2, T], f16, name="yT")
        wp_sb = persist.tile([P, 2, C], f16, name="wps")
        cst_sb = persist.tile([P, CSTW], f16, name="csts")
        dsb = persist.tile([P, 512], f16, name="dsb")
        # x is t-quarter-major and the qk weights co-major so every DMA
        # lands with 2-8KB contiguous runs on BOTH sides (small runs
        # measured ~8 B/ns per DMA engine; 4KB runs ~21 B/ns)
        xs_sb = persist.tile([P, 4, CC, 512], f16, name="xss")
        wq_sb = persist.tile([P, 2, CC, P], f16, name="wqs")
        wk_sb = persist.tile([P, 2, CC, P], f16, name="wks")
        wv_sb = persist.tile([P, CC, HD], f16, name="wvs")

        tri_v = cst_sb[:, OFF_TRI:OFF_TRI + P]
        sel_v = cst_sb[:, OFF_SEL:OFF_SEL + P]
        bq_v = cst_sb[:, OFF_BQ:OFF_BQ + 2]
        bk_v = cst_sb[:, OFF_BK:OFF_BK + 2]
        bv_v = cst_sb[:, OFF_BV:OFF_BV + HD].rearrange(
            "p (hp hi d) -> p hp hi d", hi=2, d=D
        )

        ph2 = ctx.enter_context(tc.tile_pool(name="ph2", bufs=4))
        ph3 = ctx.enter_context(tc.tile_pool(name="ph3", bufs=3))
        ps2s = ctx.enter_context(tc.tile_pool(name="ps2s", bufs=2, space="PSUM"))
        ps2a = ctx.enter_context(tc.tile_pool(name="ps2a", bufs=2, space="PSUM"))

        # one-time inits (no inputs needed)
        nc.vector.memset(dsb[:, :], 0.0)
        # constant regions of the padded v operand: zeros + the ones column
        # that makes each AV matmul emit the softmax denominator row
        nc.vector.memset(v_sb[:, :, :, 0, D + 1:P], 0.0)
        nc.vector.memset(v_sb[:, :, :, 1, 1:D], 0.0)
        nc.vector.memset(v_sb[:, :, :, 0, D:D + 1], 1.0)
        nc.vector.memset(v_sb[:, :, :, 1, 0:1], 1.0)

        # ---------------- DMA, first-needed-first ----------------
        # first x quarter rides 4 trigger queues so the first matmul group
        # (which contracts over all 8 cc chunks) unblocks as early as possible
        nc.sync.dma_start(xs_sb[:, 0, 0:3, :], xq[:, 0, 0:3, :])
        nc.gpsimd.dma_start(xs_sb[:, 0, 3:7, :], xq[:, 0, 3:7, :])
        nc.scalar.dma_start(wq_sb[:, 0, :, :], wq2[:, 0, :, :])
        nc.scalar.dma_start(xs_sb[:, 0, 7:8, :], xq[:, 0, 7:8, :])
        nc.gpsimd.dma_start(wk_sb[:, 0, :, :], wk2[:, 0, :, :])
        nc.sync.dma_start(cst_sb[:], cst)
        nc.sync.dma_start(wq_sb[:, 1, :, :], wq2[:, 1, :, :])
        nc.gpsimd.dma_start(wk_sb[:, 1, :, :], wk2[:, 1, :, :])
        nc.gpsimd.dma_start(wv_sb[:], wv2)
        for tq in range(1, 4):
            nc.sync.dma_start(xs_sb[:, tq, 0:4, :], xq[:, tq, 0:4, :])
            nc.gpsimd.dma_start(xs_sb[:, tq, 4:8, :], xq[:, tq, 4:8, :])
        nc.gpsimd.dma_start(wp_sb[:], wp2)

        # ---------------- phase 1 units ----------------
        qk_tiles = {}

        def emit_qk_half(W_s, bco, dest, co, tsl, half):
            key = (id(dest), co, tsl)
            if half == 0:
                qk_tiles[key] = ps2s.tile(
                    [P, 2, 512], f32, tag="s", name=f"qkp{co}_{tsl}"
                )
            ps = qk_tiles[key]
            for cc in range(4 * half, 4 * half + 4):
                nc.tensor.matmul(
                    ps[:, 0, :],
                    W_s[:, co, cc, :],
                    xs_sb[:, tsl, cc, :],
                    start=(cc == 0),
                    stop=(cc == CC - 1),
                )
            if half == 1:
                nc.vector.tensor_tensor(
                    dest[:, co, tsl * 512:(tsl + 1) * 512],
                    ps[:, 0, :],
                    bco.to_broadcast([P, 512]),
                    ALU.add,
                )
                del qk_tiles[key]

        def emit_qk(W_s, bco, dest, co, tsl):
            emit_qk_half(W_s, bco, dest, co, tsl, 0)
            emit_qk_half(W_s, bco, dest, co, tsl, 1)

        def emit_v(tj):
            ps = ps2s.tile([P, 2, 512], f32, tag="s")
            for cc in range(CC):
                nc.tensor.matmul(
                    ps[:, 0, 0:HD],
                    xs_sb[:, tj // 4, cc, (tj % 4) * P:(tj % 4 + 1) * P],
                    wv_sb[:, cc, :],
                    start=(cc == 0),
                    stop=(cc == CC - 1),
                )
            psv = ps[:, 0, 0:HD].rearrange("p (hp hi d) -> p hp hi d", hi=2, d=D)
            nc.vector.tensor_tensor(
                v_sb[:, tj, :, 0, 0:D], psv[:, :, 0, :], bv_v[:, :, 0, :], ALU.add
            )
            nc.vector.tensor_tensor(
                v_sb[:, tj, :, 1, D:P], psv[:, :, 1, :], bv_v[:, :, 1, :], ALU.add
            )

        # main phase 1: t-quarters 0-2 of qT/kT + v chunks 0-11. The rest
        # (quarter 3 + v 12-15) becomes attention-phase PE filler. co-outer
        # order so the second group (wk co=0, own DMA queue) never waits on
        # the wq co=1 transfer.
        for tsl in range(1):
            for co in range(2):
                for W_s, boff, dest in (
                    (wq_sb, OFF_BQ, qT_sb),
                    (wk_sb, OFF_BK, kT_sb),
                ):
                    emit_qk(
                        W_s, cst_sb[:, boff + co:boff + co + 1], dest, co, tsl
                    )
            for tj in range(4 * tsl, 4 * tsl + 4):
                emit_v(tj)

        # quarters 2-3 of phase 1 are filler inside the attention phase (the
        # PE runs ~640ns/chunk vs ACT's ~1.1us/chunk there); tagged with the
        # t-quarter so blocks that need them can force-drain first
        ph1_fill = []
        for tsl in (1, 2, 3):
            for co in range(2):
                for W_s, boff, dest in (
                    (wk_sb, OFF_BK, kT_sb),
                    (wq_sb, OFF_BQ, qT_sb),
                ):
                    for half in range(2):
                        ph1_fill.append((tsl, "qkh", lambda
                            W_s=W_s, boff=boff, dest=dest, co=co, tsl=tsl,
                            half=half: emit_qk_half(
                                W_s, cst_sb[:, boff + co:boff + co + 1],
                                dest, co, tsl, half,
                            )))
            for tj in range(4 * tsl, 4 * tsl + 4):
                ph1_fill.append((tsl, "v", lambda tj=tj: emit_v(tj)))
        proj_fill = []

        # ---------------- phase 2: attention + woven proj ----------------
        ot_tiles = {}

        def emit_proj_half(tj, co):
            if co == 0:
                ot_tiles[tj] = ph3.tile([P, C], f16, tag="ot", name=f"ot{tj}")
            ot = ot_tiles[tj]
            pps = ps2s.tile([P, 2, 512], f32, tag="s")
            for dc in range(2):
                nc.tensor.matmul(
                    pps[:, co, :],
                    yT_sb[:, dc, tj * P:(tj + 1) * P],
                    wp_sb[:, dc, co * 512:(co + 1) * 512],
                    start=(dc == 0),
                    stop=(dc == 1),
                )
            # last i-chunk: ACT has no exp left, split the copies so the
            # tail drains twice as fast
            if tj >= 12 and co == 1:
                nc.scalar.copy(ot[:, co * 512:(co + 1) * 512], pps[:, co, :])
            else:
                nc.vector.tensor_copy(
                    ot[:, co * 512:(co + 1) * 512], pps[:, co, :]
                )
            if co == 1:
                nc.sync.dma_start(out[tj * P:(tj + 1) * P, :], ot[:])
                del ot_tiles[tj]

        # emitted-work accounting (ns) to pace fillers: the PE stream should
        # stay at least as long as the ACT (exp) stream it depends on
        clk = {"pe": 0.0, "act": 0.0}

        def fill(n):
            for _ in range(n):
                if ph1_fill:
                    q, kind, fn = ph1_fill.pop(0)
                    fn()
                    clk["pe"] += 853.0
                elif proj_fill:
                    proj_fill.pop(0)()
                    clk["pe"] += 450.0
                else:
                    return

        def fill_to_rate():
            while (ph1_fill or proj_fill) and clk["act"] > clk["pe"]:
                fill(1)

        def drain_ph1(upto_quarter):
            while ph1_fill and ph1_fill[0][0] <= upto_quarter:
                q, kind, fn = ph1_fill.pop(0)
                fn()

        def emit_norm(hp, i0, av0, av1):
            # den rows were already copied into dsb right after the block's
            # last AV; broadcast + reciprocal + scale into yT
            bps = ps2s.tile([P, 2, 512], f32, tag="s")
            nc.tensor.matmul(
                bps[:, 0, :], sel_v, dsb[:],
                start=True, stop=True, skip_group_check=True,
            )
            rec = ph2.tile([P, 512], f32, tag="rec")
            nc.vector.reciprocal_approx_fast(rec[:, :], bps[:, 0, :])
            nc.vector.tensor_tensor(
                yT_sb[0:D, hp, i0:i0 + 512], av0[0:D, :], rec[0:D, :], ALU.mult
            )
            nc.vector.tensor_tensor(
                yT_sb[D:P, hp, i0:i0 + 512], av1[D:P, :], rec[D:P, :], ALU.mult
            )
            if hp == 1:
                ci = i0 // 512
                for tj in range(4 * ci, 4 * ci + 4):
                    for co in range(2):
                        proj_fill.append(
                            lambda tj=tj, co=co: emit_proj_half(tj, co)
                        )

        pending = None
        for ci in range(IC):
            i0 = ci * 512
            njc = 4 * (ci + 1)
            if ci >= 1:
                # phase-1 stragglers must land before the blocks that read
                # them (quarter ci for the kT/qT/v this i-chunk touches)
                drain_ph1(ci)
            for hp in range(2):
                av0 = ps2a.tile([P, 512], f32, tag="av0")
                av1 = ps2a.tile([P, 512], f32, tag="av1")

                def emit_s(jc):
                    diag = jc >= 4 * ci
                    o = (jc - 4 * ci) if diag else 0
                    c0 = o * P
                    sps = ps2s.tile([P, 2, 512], f32, tag="s")
                    for hi in range(2):
                        bp = D * hi
                        nc.tensor.matmul(
                            sps[:, hi, c0:512],
                            kT_sb[bp:bp + D, hp, jc * P:(jc + 1) * P],
                            qT_sb[bp:bp + D, hp, i0 + c0:i0 + 512],
                            start=True,
                            stop=True,
                            skip_group_check=True,
                        )
                    ex = ph2.tile([P, 2, 512], f16, tag="ex")
                    nc.scalar.activation(
                        ex[:, :, c0:512],
                        sps[:, :, c0:512],
                        ACTF.Exp,
                        scale=float(D) ** -0.5,
                    )
                    if diag:
                        nc.vector.tensor_tensor(
                            ex[:, :, c0:c0 + P],
                            ex[:, :, c0:c0 + P],
                            tri_v[:, None, :].to_broadcast([P, 2, P]),
                            ALU.mult,
                        )
                    return ex, c0

                def emit_av(jc, ex, c0):
                    for hi, av in ((0, av0), (1, av1)):
                        nc.tensor.matmul(
                            av[:, c0:512],
                            v_sb[:, jc, hp, hi, :],
                            ex[:, hi, c0:512],
                            start=(jc == 0),
                            stop=(jc == njc - 1),
                            skip_group_check=True,
                        )

                # score jc+1 issues before AV jc so the in-order PE stream
                # never waits on the ACT exp; fillers pad the PE to ACT rate;
                # the previous block's normalization lands at jc==3, behind a
                # filler, so its broadcast-matmul never waits on the den rows
                pend_av = None
                for jc in range(njc):
                    ex, c0 = emit_s(jc)
                    w = 512 - (jc - 4 * ci) * P if jc >= 4 * ci else 512
                    clk["pe"] += w / 2.4 + 8
                    clk["act"] += 2 * w / 1.2 + 240
                    if jc >= 4 * ci:
                        clk["act"] += 300
                    # fillers go between the score and the AV that depends on
                    # the previous chunk's exp, absorbing ACT-rate jitter
                    if (ci, hp) != (0, 0):
                        fill_to_rate()
                    if pend_av is not None:
                        emit_av(*pend_av)
                        clk["pe"] += 2 * (512 - pend_av[2]) / 2.4 + 16
                    pend_av = (jc, ex, c0)
                    if jc == 3 and pending is not None:
                        emit_norm(*pending)
                        pending = None
                        clk["pe"] += 230.0
                emit_av(*pend_av)
                clk["pe"] += 2 * (512 - pend_av[2]) / 2.4 + 16
                # den rows -> staging now; the rest of the normalization is
                # emitted inside the next block so the PE never stalls on it
                nc.vector.tensor_copy(dsb[D:D + 1, :], av0[D:D + 1, :])
                nc.vector.tensor_copy(dsb[0:1, :], av1[0:1, :])
                pending = (hp, i0, av0, av1)
        fill(1)
        emit_norm(*pending)
        while proj_fill:
            proj_fill.pop(0)()
    nc.compile()
    return nc


def _get_nc():
    global _NC
    if _NC is None:
        _NC = _build_nc()
    return _NC


def _pack_inputs(x_b, W_qkv, b_qkv, W_proj, g):
    """Host-side packing for core (batch, head-group g): fp16, DMA-friendly."""
    f16 = np.float16
    s0 = HD * g
    xt = np.ascontiguousarray(x_b.T).astype(f16)          # [C, T]
    xqa = np.ascontiguousarray(
        xt.reshape(CC, P, 4, 512).transpose(1, 2, 0, 3)   # [p, quarter, o, t]
    )

    def wpack(col0):
        w = W_qkv[:, col0:col0 + HD].astype(f16)          # [C, HD]
        return np.ascontiguousarray(w.reshape(CC, P, 2, P).transpose(1, 2, 0, 3))

    wv_ = W_qkv[:, 2 * C + s0:2 * C + s0 + HD].astype(f16)
    wv_p = np.ascontiguousarray(wv_.reshape(CC, P, HD).transpose(1, 0, 2))
    wp_ = W_proj[s0:s0 + HD, :].astype(f16)               # [HD, C]
    wp_p = np.ascontiguousarray(wp_.reshape(2, P, C).transpose(1, 0, 2))

    cstm = np.zeros((P, CSTW), dtype=f16)
    cstm[:, OFF_TRI:OFF_TRI + P] = np.triu(np.ones((P, P), dtype=f16))
    cstm[D, OFF_SEL:OFF_SEL + D] = 1.0
    cstm[0, OFF_SEL + D:OFF_SEL + P] = 1.0
    cstm[:, OFF_BQ:OFF_BQ + 2] = b_qkv[s0:s0 + HD].reshape(2, P).T
    cstm[:, OFF_BK:OFF_BK + 2] = (
        b_qkv[C + s0:C + s0 + HD].reshape(2, P).T
    )
    cstm[:, OFF_BV:OFF_BV + HD] = b_qkv[2 * C + s0:2 * C + s0 + HD]

    return {
        "xq": xqa,
        "wq2": wpack(s0),
        "wk2": wpack(C + s0),
        "wv2": wv_p,
        "wp2": wp_p,
        "cst": np.ascontiguousarray(cstm),
    }


def kernel(x, W_qkv, b_qkv, W_proj, b_proj):
    global LAST_RESULTS
    from concourse import bass_utils

    x = np.asarray(x, dtype=np.float32)
    W_qkv = np.asarray(W_qkv, dtype=np.float32)
    b_qkv = np.asarray(b_qkv, dtype=np.float32)
    W_proj = np.asarray(W_proj, dtype=np.float32)
    b_proj = np.asarray(b_proj, dtype=np.float32)

    nc = _get_nc()
    in_maps = []
    for c in range(8):
        b, g = divmod(c, 4)
        in_maps.append(_pack_inputs(x[b], W_qkv, b_qkv, W_proj, g))

    res = bass_utils.run_bass_kernel_spmd(nc, in_maps, core_ids=list(range(8)))
    LAST_RESULTS = res
    ys = []
    for b in range(2):
        y = res.results[4 * b]["out"].astype(np.float64)
        for g in range(1, 4):
            y = y + res.results[4 * b + g]["out"]
        ys.append((y + b_proj).astype(np.float32))
    return np.stack(ys, axis=0)


# revision 16
# speedup vs baseline: 1.0053x; 1.0053x over previous
"""Causal self-attention (B=2, T=2048, C=1024, H=16) on 8 Trainium2 cores.

Sharding: data-parallel over batch (2) x tensor-parallel over heads (4 groups
of 4 heads). Core c handles batch b = c//4, head group g = c%4 (heads 4g..4g+3).
Each core computes its qkv column slice, full causal TxT attention for its 4
heads, and a partial row-parallel projection. Host sums the 4 partial proj
outputs per batch and adds b_proj.

Device kernel layout notes (v3):
- all matmul operands are fp16: on TRN2 fp32r runs fp32_mode=HIGH (2 PE
  passes -> 2 cycles/row, doubled LDWEIGHTS, and the row-bank conflict
  serializes the two K=64 head-quadrant score matmuls). fp16 is 1 cycle/row,
  enables FWL weight loads, and the hi=0/hi=1 score matmuls (stationary rows
  0-63 / 64-127) genuinely overlap (measured dstart ~4ns). PSUM stays fp32.
- feature-major ("transposed") layouts throughout: qT/kT [d, t] so PE
  contraction dims line up with no on-device transposes
- host pre-packs every DRAM tensor so each DMA moves 2-8KB contiguous
  per-partition lines (small strided lines measured ~85GB/s/queue; packed
  ~200+GB/s), and the first-needed tensors (wq col group 0, x t-quarter 0)
  are triggered first; all small constants ship as one [128, 516] f16 blob
- softmax without max-subtraction (logits ~N(0,1), exp fits fp16); exp runs
  on the ACT engine - the second binding resource (~70us) after the PE
  (~100us); the AV stationary operand embeds an all-ones column so each AV
  matmul also emits the softmax denominator row for free
- denominator rows are partition-broadcast with a single K=128 matmul against
  a constant selector matrix (col j<64 reads row 64 = den0, col j>=64 reads
  row 0 = den1) over a pre-zeroed staging row-pair; reciprocal via the
  custom-DVE reciprocal_approx_fast (~5x faster than exact reciprocal)
- causal masking: upper-triangle j-chunks skipped; diagonal chunks narrow the
  score/exp/AV column range to [o*128, 512) and one [128,128] triangular
  multiplicative mask handles the partial strip
- scheduling: the score matmul for chunk jc+1 issues before the AV for jc so
  the in-order PE stream never waits on ACT; "filler" PE work (projection of
  finished i-chunks + deferred phase-1 work: the last t-quarter of qT/kT and
  v chunks 12-15) is woven in at a fixed cadence to keep the PE at the ACT
  rate. Dense PE occupancy also parks the HAM clock gate at 2.4 GHz (any
  ~3.4us idle window re-throttles the PE to 1.2 GHz).
- normalization lags one block: den-row copies (DVE) issue right after the
  block's last AV, the broadcast-matmul + reciprocal + yT scale are emitted
  inside the NEXT block so the PE never stalls on DVE latency
- PSUM budget (8 banks): "s" tag [128,2,512] bufs=2 (scores, proj, den
  broadcast, and phase-1 qkv groups all share it) + av0/av1 bufs=2 each
- psum->sbuf copies for the projection ride the GpSimd engine (DVE is the
  #3 resource); output tiles stream out per-tile as fp16 on the sync queue
"""

import os
import sys

sys.path.insert(0, "/opt/trn_rl_repo")

import numpy as np

P = 128
T = 2048
C = 1024
D = 64
HPC = 4          # heads per core
HD = HPC * D     # 256 qkv columns per core
CC = C // P      # 8 contraction chunks
TC = T // P      # 16 t-chunks of 128
IC = T // 512    # 4 i-chunks of 512

# const blob column offsets
OFF_TRI = 0
OFF_SEL = 128
OFF_BQ = 256
OFF_BK = 258
OFF_BV = 260
CSTW = 516

_NC = None
LAST_RESULTS = None


def _build_nc():
    import concourse.mybir as mybir
    import concourse.tile as tile
    from concourse import bacc
    from contextlib import ExitStack

    dt = mybir.dt
    f32 = dt.float32
    f16 = dt.float16
    ALU = mybir.AluOpType
    ACTF = mybir.ActivationFunctionType

    nc = bacc.Bacc(
        "TRN2",
        target_bir_lowering=False,
        debug=False,
        enable_asserts=False,
        num_devices=8,
    )

    # host-packed layouts: contiguous per-partition lines per transfer
    xq = nc.dram_tensor("xq", [P, 4, CC, 512], f16, kind="ExternalInput").ap()
    wq2 = nc.dram_tensor("wq2", [P, 2, CC, P], f16, kind="ExternalInput").ap()
    wk2 = nc.dram_tensor("wk2", [P, 2, CC, P], f16, kind="ExternalInput").ap()
    wv2 = nc.dram_tensor("wv2", [P, CC, HD], f16, kind="ExternalInput").ap()
    wp2 = nc.dram_tensor("wp2", [P, 2, C], f16, kind="ExternalInput").ap()
    cst = nc.dram_tensor("cst", [P, CSTW], f16, kind="ExternalInput").ap()
    out = nc.dram_tensor("out", [T, C], f16, kind="ExternalOutput").ap()

    with tile.TileContext(nc) as tc, ExitStack() as ctx:
        persist = ctx.enter_context(tc.tile_pool(name="persist", bufs=1))
        qT_sb = persist.tile([P, 2, T], f16, name="qT")    # [d%128, dchunk, t]
        kT_sb = persist.tile([P, 2, T], f16, name="kT")
        v_sb = persist.tile([P, TC, 2, 2, P], f16, name="v")  # [t%128, tchunk, hpair, hi, 128-padded d]
        yT_sb = persist.tile([P, 2, T], f16, name="yT")
        wp_sb = persist.tile([P, 2, C], f16, name="wps")
        cst_sb = persist.tile([P, CSTW], f16, name="csts")
        dsb = persist.tile([P, 512], f16, name="dsb")
        # x is t-quarter-major and the qk weights co-major so every DMA
        # lands with 2-8KB contiguous runs on BOTH sides (small runs
        # measured ~8 B/ns per DMA engine; 4KB runs ~21 B/ns)
        xs_sb = persist.tile([P, 4, CC, 512], f16, name="xss")
        wq_sb = persist.tile([P, 2, CC, P], f16, name="wqs")
        wk_sb = persist.tile([P, 2, CC, P], f16, name="wks")
        wv_sb = persist.tile([P, CC, HD], f16, name="wvs")

        tri_v = cst_sb[:, OFF_TRI:OFF_TRI + P]
        sel_v = cst_sb[:, OFF_SEL:OFF_SEL + P]
        bq_v = cst_sb[:, OFF_BQ:OFF_BQ + 2]
        bk_v = cst_sb[:, OFF_BK:OFF_BK + 2]
        bv_v = cst_sb[:, OFF_BV:OFF_BV + HD].rearrange(
            "p (hp hi d) -> p hp hi d", hi=2, d=D
        )

        ph2 = ctx.enter_context(tc.tile_pool(name="ph2", bufs=4))
        ph3 = ctx.enter_context(tc.tile_pool(name="ph3", bufs=3))
        ps2s = ctx.enter_context(tc.tile_pool(name="ps2s", bufs=2, space="PSUM"))
        ps2a = ctx.enter_context(tc.tile_pool(name="ps2a", bufs=2, space="PSUM"))

        # one-time inits (no inputs needed)
        nc.vector.memset(dsb[:, :], 0.0)
        # constant regions of the padded v operand: zeros + the ones column
        # that makes each AV matmul emit the softmax denominator row
        nc.vector.memset(v_sb[:, :, :, 0, D + 1:P], 0.0)
        nc.vector.memset(v_sb[:, :, :, 1, 1:D], 0.0)
        nc.vector.memset(v_sb[:, :, :, 0, D:D + 1], 1.0)
        nc.vector.memset(v_sb[:, :, :, 1, 0:1], 1.0)

        # ---------------- DMA, first-needed-first ----------------
        # first x quarter rides 4 trigger queues so the first matmul group
        # (which contracts over all 8 cc chunks) unblocks as early as possible
        nc.sync.dma_start(xs_sb[:, 0, 0:3, :], xq[:, 0, 0:3, :])
        nc.gpsimd.dma_start(xs_sb[:, 0, 3:7, :], xq[:, 0, 3:7, :])
        nc.scalar.dma_start(wq_sb[:, 0, :, :], wq2[:, 0, :, :])
        nc.scalar.dma_start(xs_sb[:, 0, 7:8, :], xq[:, 0, 7:8, :])
        nc.gpsimd.dma_start(wk_sb[:, 0, :, :], wk2[:, 0, :, :])
        nc.sync.dma_start(cst_sb[:], cst)
        nc.sync.dma_start(wq_sb[:, 1, :, :], wq2[:, 1, :, :])
        nc.gpsimd.dma_start(wk_sb[:, 1, :, :], wk2[:, 1, :, :])
        nc.gpsimd.dma_start(wv_sb[:], wv2)
        for tq in range(1, 4):
            nc.sync.dma_start(xs_sb[:, tq, 0:4, :], xq[:, tq, 0:4, :])
            nc.gpsimd.dma_start(xs_sb[:, tq, 4:8, :], xq[:, tq, 4:8, :])
        nc.gpsimd.dma_start(wp_sb[:], wp2)

        # ---------------- phase 1 units ----------------
        qk_tiles = {}

        def emit_qk_half(W_s, bco, dest, co, tsl, half):
            key = (id(dest), co, tsl)
            if half == 0:
                qk_tiles[key] = ps2s.tile(
                    [P, 2, 512], f32, tag="s", name=f"qkp{co}_{tsl}"
                )
            ps = qk_tiles[key]
            for cc in range(4 * half, 4 * half + 4):
                nc.tensor.matmul(
                    ps[:, 0, :],
                    W_s[:, co, cc, :],
                    xs_sb[:, tsl, cc, :],
                    start=(cc == 0),
                    stop=(cc == CC - 1),
                )
            if half == 1:
                nc.vector.tensor_tensor(
                    dest[:, co, tsl * 512:(tsl + 1) * 512],
                    ps[:, 0, :],
                    bco.to_broadcast([P, 512]),
                    ALU.add,
                )
                del qk_tiles[key]

        def emit_qk(W_s, bco, dest, co, tsl):
            emit_qk_half(W_s, bco, dest, co, tsl, 0)
            emit_qk_half(W_s, bco, dest, co, tsl, 1)

        def emit_v(tj):
            ps = ps2s.tile([P, 2, 512], f32, tag="s")
            for cc in range(CC):
                nc.tensor.matmul(
                    ps[:, 0, 0:HD],
                    xs_sb[:, tj // 4, cc, (tj % 4) * P:(tj % 4 + 1) * P],
                    wv_sb[:, cc, :],
                    start=(cc == 0),
                    stop=(cc == CC - 1),
                )
            psv = ps[:, 0, 0:HD].rearrange("p (hp hi d) -> p hp hi d", hi=2, d=D)
            nc.vector.tensor_tensor(
                v_sb[:, tj, :, 0, 0:D], psv[:, :, 0, :], bv_v[:, :, 0, :], ALU.add
            )
            nc.vector.tensor_tensor(
                v_sb[:, tj, :, 1, D:P], psv[:, :, 1, :], bv_v[:, :, 1, :], ALU.add
            )

        # main phase 1: t-quarters 0-2 of qT/kT + v chunks 0-11. The rest
        # (quarter 3 + v 12-15) becomes attention-phase PE filler. co-outer
        # order so the second group (wk co=0, own DMA queue) never waits on
        # the wq co=1 transfer.
        for tsl in range(1):
            for co in range(2):
                for W_s, boff, dest in (
                    (wq_sb, OFF_BQ, qT_sb),
                    (wk_sb, OFF_BK, kT_sb),
                ):
                    emit_qk(
                        W_s, cst_sb[:, boff + co:boff + co + 1], dest, co, tsl
                    )
            for tj in range(4 * tsl, 4 * tsl + 4):
                emit_v(tj)

        # quarters 2-3 of phase 1 are filler inside the attention phase (the
        # PE runs ~640ns/chunk vs ACT's ~1.1us/chunk there); tagged with the
        # t-quarter so blocks that need them can force-drain first
        ph1_fill = []
        for tsl in (1, 2, 3):
            for W_s, boff, dest, kind in (
                (wq_sb, OFF_BQ, qT_sb, "q"),
                (wk_sb, OFF_BK, kT_sb, "k"),
            ):
                for co in range(2):
                    for half in range(2):
                        ph1_fill.append((tsl, kind, lambda
                            W_s=W_s, boff=boff, dest=dest, co=co, tsl=tsl,
                            half=half: emit_qk_half(
                                W_s, cst_sb[:, boff + co:boff + co + 1],
                                dest, co, tsl, half,
                            )))
            for tj in range(4 * tsl, 4 * tsl + 4):
                ph1_fill.append((tsl, "v", lambda tj=tj: emit_v(tj)))
        proj_fill = []

        # ---------------- phase 2: attention + woven proj ----------------
        ot_tiles = {}

        def emit_proj_half(tj, co):
            if co == 0:
                ot_tiles[tj] = ph3.tile([P, C], f16, tag="ot", name=f"ot{tj}")
            ot = ot_tiles[tj]
            pps = ps2s.tile([P, 2, 512], f32, tag="s")
            for dc in range(2):
                nc.tensor.matmul(
                    pps[:, co, :],
                    yT_sb[:, dc, tj * P:(tj + 1) * P],
                    wp_sb[:, dc, co * 512:(co + 1) * 512],
                    start=(dc == 0),
                    stop=(dc == 1),
                )
            # last i-chunk: ACT has no exp left, split the copies so the
            # tail drains twice as fast
            if tj >= 12 and co == 1:
                nc.scalar.copy(ot[:, co * 512:(co + 1) * 512], pps[:, co, :])
            else:
                nc.vector.tensor_copy(
                    ot[:, co * 512:(co + 1) * 512], pps[:, co, :]
                )
            if co == 1:
                nc.sync.dma_start(out[tj * P:(tj + 1) * P, :], ot[:])
                del ot_tiles[tj]

        # emitted-work accounting (ns) to pace fillers: the PE stream should
        # stay at least as long as the ACT (exp) stream it depends on
        clk = {"pe": 0.0, "act": 0.0}

        def fill(n):
            for _ in range(n):
                if ph1_fill:
                    q, kind, fn = ph1_fill.pop(0)
                    fn()
                    clk["pe"] += 853.0
                elif proj_fill:
                    proj_fill.pop(0)()
                    clk["pe"] += 450.0
                else:
                    return

        def fill_to_rate():
            while (ph1_fill or proj_fill) and clk["act"] > clk["pe"]:
                fill(1)

        def drain_ph1(upto_quarter, kinds=("q", "k", "v")):
            i = 0
            while i < len(ph1_fill):
                q, kind, fn = ph1_fill[i]
                if q <= upto_quarter and kind in kinds:
                    ph1_fill.pop(i)
                    fn()
                else:
                    i += 1

        def emit_norm(hp, i0, av0, av1):
            # den rows were already copied into dsb right after the block's
            # last AV; broadcast + reciprocal + scale into yT
            bps = ps2s.tile([P, 2, 512], f32, tag="s")
            nc.tensor.matmul(
                bps[:, 0, :], sel_v, dsb[:],
                start=True, stop=True, skip_group_check=True,
            )
            rec = ph2.tile([P, 512], f32, tag="rec")
            nc.vector.reciprocal_approx_fast(rec[:, :], bps[:, 0, :])
            nc.vector.tensor_tensor(
                yT_sb[0:D, hp, i0:i0 + 512], av0[0:D, :], rec[0:D, :], ALU.mult
            )
            nc.vector.tensor_tensor(
                yT_sb[D:P, hp, i0:i0 + 512], av1[D:P, :], rec[D:P, :], ALU.mult
            )
            if hp == 1:
                ci = i0 // 512
                for tj in range(4 * ci, 4 * ci + 4):
                    for co in range(2):
                        proj_fill.append(
                            lambda tj=tj, co=co: emit_proj_half(tj, co)
                        )

        pending = None
        for ci in range(IC):
            i0 = ci * 512
            njc = 4 * (ci + 1)
            if ci >= 1:
                # only this i-chunk's qT is needed before the block starts;
                # its kT/v stragglers can drain any time before the diagonal
                drain_ph1(ci, kinds=("q",))
            for hp in range(2):
                av0 = ps2a.tile([P, 512], f32, tag="av0")
                av1 = ps2a.tile([P, 512], f32, tag="av1")

                def emit_s(jc):
                    diag = jc >= 4 * ci
                    o = (jc - 4 * ci) if diag else 0
                    c0 = o * P
                    sps = ps2s.tile([P, 2, 512], f32, tag="s")
                    for hi in range(2):
                        bp = D * hi
                        nc.tensor.matmul(
                            sps[:, hi, c0:512],
                            kT_sb[bp:bp + D, hp, jc * P:(jc + 1) * P],
                            qT_sb[bp:bp + D, hp, i0 + c0:i0 + 512],
                            start=True,
                            stop=True,
                            skip_group_check=True,
                        )
                    ex = ph2.tile([P, 2, 512], f16, tag="ex")
                    nc.scalar.activation(
                        ex[:, :, c0:512],
                        sps[:, :, c0:512],
                        ACTF.Exp,
                        scale=float(D) ** -0.5,
                    )
                    if diag:
                        nc.vector.tensor_tensor(
                            ex[:, :, c0:c0 + P],
                            ex[:, :, c0:c0 + P],
                            tri_v[:, None, :].to_broadcast([P, 2, P]),
                            ALU.mult,
                        )
                    return ex, c0

                def emit_av(jc, ex, c0):
                    for hi, av in ((0, av0), (1, av1)):
                        nc.tensor.matmul(
                            av[:, c0:512],
                            v_sb[:, jc, hp, hi, :],
                            ex[:, hi, c0:512],
                            start=(jc == 0),
                            stop=(jc == njc - 1),
                            skip_group_check=True,
                        )

                # score jc+1 issues before AV jc so the in-order PE stream
                # never waits on the ACT exp; fillers pad the PE to ACT rate;
                # the previous block's normalization lands at jc==3, behind a
                # filler, so its broadcast-matmul never waits on the den rows
                pend_av = None
                for jc in range(njc):
                    if ci >= 1 and jc == 4 * ci:
                        drain_ph1(ci)
                    ex, c0 = emit_s(jc)
                    w = 512 - (jc - 4 * ci) * P if jc >= 4 * ci else 512
                    clk["pe"] += w / 2.4 + 8
                    clk["act"] += 2 * w / 1.2 + 240
                    if jc >= 4 * ci:
                        clk["act"] += 300
                    # fillers go between the score and the AV that depends on
                    # the previous chunk's exp, absorbing ACT-rate jitter
                    if (ci, hp) != (0, 0):
                        fill_to_rate()
                    if pend_av is not None:
                        emit_av(*pend_av)
                        clk["pe"] += 2 * (512 - pend_av[2]) / 2.4 + 16
                    pend_av = (jc, ex, c0)
                    if jc == 3 and pending is not None:
                        emit_norm(*pending)
                        pending = None
                        clk["pe"] += 230.0
                emit_av(*pend_av)
                clk["pe"] += 2 * (512 - pend_av[2]) / 2.4 + 16
                # den rows -> staging now; the rest of the normalization is
                # emitted inside the next block so the PE never stalls on it
                nc.vector.tensor_copy(dsb[D:D + 1, :], av0[D:D + 1, :])
                nc.vector.tensor_copy(dsb[0:1, :], av1[0:1, :])
                pending = (hp, i0, av0, av1)
        fill(1)
        emit_norm(*pending)
        while proj_fill:
            proj_fill.pop(0)()
    nc.compile()
    return nc


def _get_nc():
    global _NC
    if _NC is None:
        _NC = _build_nc()
    return _NC


def _pack_inputs(x_b, W_qkv, b_qkv, W_proj, g):
    """Host-side packing for core (batch, head-group g): fp16, DMA-friendly."""
    f16 = np.float16
    s0 = HD * g
    xt = np.ascontiguousarray(x_b.T).astype(f16)          # [C, T]
    xqa = np.ascontiguousarray(
        xt.reshape(CC, P, 4, 512).transpose(1, 2, 0, 3)   # [p, quarter, o, t]
    )

    def wpack(col0):
        w = W_qkv[:, col0:col0 + HD].astype(f16)          # [C, HD]
        return np.ascontiguousarray(w.reshape(CC, P, 2, P).transpose(1, 2, 0, 3))

    wv_ = W_qkv[:, 2 * C + s0:2 * C + s0 + HD].astype(f16)
    wv_p = np.ascontiguousarray(wv_.reshape(CC, P, HD).transpose(1, 0, 2))
    wp_ = W_proj[s0:s0 + HD, :].astype(f16)               # [HD, C]
    wp_p = np.ascontiguousarray(wp_.reshape(2, P, C).transpose(1, 0, 2))

    cstm = np.zeros((P, CSTW), dtype=f16)
    cstm[:, OFF_TRI:OFF_TRI + P] = np.triu(np.ones((P, P), dtype=f16))
    cstm[D, OFF_SEL:OFF_SEL + D] = 1.0
    cstm[0, OFF_SEL + D:OFF_SEL + P] = 1.0
    cstm[:, OFF_BQ:OFF_BQ + 2] = b_qkv[s0:s0 + HD].reshape(2, P).T
    cstm[:, OFF_BK:OFF_BK + 2] = (
        b_qkv[C + s0:C + s0 + HD].reshape(2, P).T
    )
    cstm[:, OFF_BV:OFF_BV + HD] = b_qkv[2 * C + s0:2 * C + s0 + HD]

    return {
        "xq": xqa,
        "wq2": wpack(s0),
        "wk2": wpack(C + s0),
        "wv2": wv_p,
        "wp2": wp_p,
        "cst": np.ascontiguousarray(cstm),
    }


def kernel(x, W_qkv, b_qkv, W_proj, b_proj):
    global LAST_RESULTS
    from concourse import bass_utils

    x = np.asarray(x, dtype=np.float32)
    W_qkv = np.asarray(W_qkv, dtype=np.float32)
    b_qkv = np.asarray(b_qkv, dtype=np.float32)
    W_proj = np.asarray(W_proj, dtype=np.float32)
    b_proj = np.asarray(b_proj, dtype=np.float32)

    nc = _get_nc()
    in_maps = []
    for c in range(8):
        b, g = divmod(c, 4)
        in_maps.append(_pack_inputs(x[b], W_qkv, b_qkv, W_proj, g))

    res = bass_utils.run_bass_kernel_spmd(nc, in_maps, core_ids=list(range(8)))
    LAST_RESULTS = res
    ys = []
    for b in range(2):
        y = res.results[4 * b]["out"].astype(np.float64)
        for g in range(1, 4):
            y = y + res.results[4 * b + g]["out"]
        ys.append((y + b_proj).astype(np.float32))
    return np.stack(ys, axis=0)


# revision 17
# speedup vs baseline: 1.0149x; 1.0095x over previous
"""Causal self-attention (B=2, T=2048, C=1024, H=16) on 8 Trainium2 cores.

Sharding: data-parallel over batch (2) x tensor-parallel over heads (4 groups
of 4 heads). Core c handles batch b = c//4, head group g = c%4 (heads 4g..4g+3).
Each core computes its qkv column slice, full causal TxT attention for its 4
heads, and a partial row-parallel projection. Host sums the 4 partial proj
outputs per batch and adds b_proj.

Device kernel layout notes (v3):
- all matmul operands are fp16: on TRN2 fp32r runs fp32_mode=HIGH (2 PE
  passes -> 2 cycles/row, doubled LDWEIGHTS, and the row-bank conflict
  serializes the two K=64 head-quadrant score matmuls). fp16 is 1 cycle/row,
  enables FWL weight loads, and the hi=0/hi=1 score matmuls (stationary rows
  0-63 / 64-127) genuinely overlap (measured dstart ~4ns). PSUM stays fp32.
- feature-major ("transposed") layouts throughout: qT/kT [d, t] so PE
  contraction dims line up with no on-device transposes
- host pre-packs every DRAM tensor so each DMA moves 2-8KB contiguous
  per-partition lines (small strided lines measured ~85GB/s/queue; packed
  ~200+GB/s), and the first-needed tensors (wq col group 0, x t-quarter 0)
  are triggered first; all small constants ship as one [128, 516] f16 blob
- softmax without max-subtraction (logits ~N(0,1), exp fits fp16); exp runs
  on the ACT engine - the second binding resource (~70us) after the PE
  (~100us); the AV stationary operand embeds an all-ones column so each AV
  matmul also emits the softmax denominator row for free
- denominator rows are partition-broadcast with a single K=128 matmul against
  a constant selector matrix (col j<64 reads row 64 = den0, col j>=64 reads
  row 0 = den1) over a pre-zeroed staging row-pair; reciprocal via the
  custom-DVE reciprocal_approx_fast (~5x faster than exact reciprocal)
- causal masking: upper-triangle j-chunks skipped; diagonal chunks narrow the
  score/exp/AV column range to [o*128, 512) and one [128,128] triangular
  multiplicative mask handles the partial strip
- scheduling: the score matmul for chunk jc+1 issues before the AV for jc so
  the in-order PE stream never waits on ACT; "filler" PE work (projection of
  finished i-chunks + deferred phase-1 work: the last t-quarter of qT/kT and
  v chunks 12-15) is woven in at a fixed cadence to keep the PE at the ACT
  rate. Dense PE occupancy also parks the HAM clock gate at 2.4 GHz (any
  ~3.4us idle window re-throttles the PE to 1.2 GHz).
- normalization lags one block: den-row copies (DVE) issue right after the
  block's last AV, the broadcast-matmul + reciprocal + yT scale are emitted
  inside the NEXT block so the PE never stalls on DVE latency
- PSUM budget (8 banks): "s" tag [128,2,512] bufs=2 (scores, proj, den
  broadcast, and phase-1 qkv groups all share it) + av0/av1 bufs=2 each
- psum->sbuf copies for the projection ride the GpSimd engine (DVE is the
  #3 resource); output tiles stream out per-tile as fp16 on the sync queue
"""

import os
import sys

sys.path.insert(0, "/opt/trn_rl_repo")

import numpy as np

P = 128
T = 2048
C = 1024
D = 64
HPC = 4          # heads per core
HD = HPC * D     # 256 qkv columns per core
CC = C // P      # 8 contraction chunks
TC = T // P      # 16 t-chunks of 128
IC = T // 512    # 4 i-chunks of 512

# const blob column offsets
OFF_TRI = 0
OFF_SEL = 128
OFF_BQ = 256
OFF_BK = 258
OFF_BV = 260
CSTW = 516

_NC = None
LAST_RESULTS = None


def _build_nc():
    import concourse.mybir as mybir
    import concourse.tile as tile
    from concourse import bacc
    from contextlib import ExitStack

    dt = mybir.dt
    f32 = dt.float32
    f16 = dt.float16
    ALU = mybir.AluOpType
    ACTF = mybir.ActivationFunctionType

    nc = bacc.Bacc(
        "TRN2",
        target_bir_lowering=False,
        debug=False,
        enable_asserts=False,
        num_devices=8,
    )

    # host-packed layouts: contiguous per-partition lines per transfer
    xq = nc.dram_tensor("xq", [P, 4, CC, 512], f16, kind="ExternalInput").ap()
    wq2 = nc.dram_tensor("wq2", [P, 2, CC, P], f16, kind="ExternalInput").ap()
    wk2 = nc.dram_tensor("wk2", [P, 2, CC, P], f16, kind="ExternalInput").ap()
    wv2 = nc.dram_tensor("wv2", [P, CC, HD], f16, kind="ExternalInput").ap()
    wp2 = nc.dram_tensor("wp2", [P, 2, C], f16, kind="ExternalInput").ap()
    cst = nc.dram_tensor("cst", [P, CSTW], f16, kind="ExternalInput").ap()
    out = nc.dram_tensor("out", [T, C], f16, kind="ExternalOutput").ap()

    with tile.TileContext(nc) as tc, ExitStack() as ctx:
        persist = ctx.enter_context(tc.tile_pool(name="persist", bufs=1))
        qT_sb = persist.tile([P, 2, T], f16, name="qT")    # [d%128, dchunk, t]
        kT_sb = persist.tile([P, 2, T], f16, name="kT")
        v_sb = persist.tile([P, TC, 2, 2, P], f16, name="v")  # [t%128, tchunk, hpair, hi, 128-padded d]
        yT_sb = persist.tile([P, 2, T], f16, name="yT")
        wp_sb = persist.tile([P, 2, C], f16, name="wps")
        cst_sb = persist.tile([P, CSTW], f16, name="csts")
        dsb = persist.tile([P, 512], f16, name="dsb")
        # x is t-quarter-major and the qk weights co-major so every DMA
        # lands with 2-8KB contiguous runs on BOTH sides (small runs
        # measured ~8 B/ns per DMA engine; 4KB runs ~21 B/ns)
        xs_sb = persist.tile([P, 4, CC, 512], f16, name="xss")
        wq_sb = persist.tile([P, 2, CC, P], f16, name="wqs")
        wk_sb = persist.tile([P, 2, CC, P], f16, name="wks")
        wv_sb = persist.tile([P, CC, HD], f16, name="wvs")

        tri_v = cst_sb[:, OFF_TRI:OFF_TRI + P]
        sel_v = cst_sb[:, OFF_SEL:OFF_SEL + P]
        bq_v = cst_sb[:, OFF_BQ:OFF_BQ + 2]
        bk_v = cst_sb[:, OFF_BK:OFF_BK + 2]
        bv_v = cst_sb[:, OFF_BV:OFF_BV + HD].rearrange(
            "p (hp hi d) -> p hp hi d", hi=2, d=D
        )

        ph2 = ctx.enter_context(tc.tile_pool(name="ph2", bufs=4))
        ph3 = ctx.enter_context(tc.tile_pool(name="ph3", bufs=3))
        ps2s = ctx.enter_context(tc.tile_pool(name="ps2s", bufs=2, space="PSUM"))
        ps2a = ctx.enter_context(tc.tile_pool(name="ps2a", bufs=2, space="PSUM"))

        # one-time inits (no inputs needed)
        nc.vector.memset(dsb[:, :], 0.0)
        # constant regions of the padded v operand: zeros + the ones column
        # that makes each AV matmul emit the softmax denominator row
        nc.vector.memset(v_sb[:, :, :, 0, D + 1:P], 0.0)
        nc.vector.memset(v_sb[:, :, :, 1, 1:D], 0.0)
        nc.vector.memset(v_sb[:, :, :, 0, D:D + 1], 1.0)
        nc.vector.memset(v_sb[:, :, :, 1, 0:1], 1.0)

        # ---------------- DMA, first-needed-first ----------------
        # first x quarter rides 4 trigger queues so the first matmul group
        # (which contracts over all 8 cc chunks) unblocks as early as possible
        nc.sync.dma_start(xs_sb[:, 0, 0:3, :], xq[:, 0, 0:3, :])
        nc.gpsimd.dma_start(xs_sb[:, 0, 3:7, :], xq[:, 0, 3:7, :])
        nc.scalar.dma_start(wq_sb[:, 0, :, :], wq2[:, 0, :, :])
        nc.scalar.dma_start(xs_sb[:, 0, 7:8, :], xq[:, 0, 7:8, :])
        nc.gpsimd.dma_start(wk_sb[:, 0, :, :], wk2[:, 0, :, :])
        nc.sync.dma_start(cst_sb[:], cst)
        nc.sync.dma_start(wq_sb[:, 1, :, :], wq2[:, 1, :, :])
        nc.gpsimd.dma_start(wk_sb[:, 1, :, :], wk2[:, 1, :, :])
        nc.gpsimd.dma_start(wv_sb[:], wv2)
        for tq in range(1, 4):
            nc.sync.dma_start(xs_sb[:, tq, 0:4, :], xq[:, tq, 0:4, :])
            nc.gpsimd.dma_start(xs_sb[:, tq, 4:8, :], xq[:, tq, 4:8, :])
        nc.gpsimd.dma_start(wp_sb[:], wp2)

        # ---------------- phase 1 units ----------------
        qk_tiles = {}

        def emit_qk_half(W_s, bco, dest, co, tsl, half):
            key = (id(dest), co, tsl)
            if half == 0:
                qk_tiles[key] = ps2s.tile(
                    [P, 2, 512], f32, tag="s", name=f"qkp{co}_{tsl}"
                )
            ps = qk_tiles[key]
            for cc in range(4 * half, 4 * half + 4):
                nc.tensor.matmul(
                    ps[:, 0, :],
                    W_s[:, co, cc, :],
                    xs_sb[:, tsl, cc, :],
                    start=(cc == 0),
                    stop=(cc == CC - 1),
                )
            if half == 1:
                nc.vector.tensor_tensor(
                    dest[:, co, tsl * 512:(tsl + 1) * 512],
                    ps[:, 0, :],
                    bco.to_broadcast([P, 512]),
                    ALU.add,
                )
                del qk_tiles[key]

        def emit_qk(W_s, bco, dest, co, tsl):
            emit_qk_half(W_s, bco, dest, co, tsl, 0)
            emit_qk_half(W_s, bco, dest, co, tsl, 1)

        def emit_v(tj):
            ps = ps2s.tile([P, 2, 512], f32, tag="s")
            for cc in range(CC):
                nc.tensor.matmul(
                    ps[:, 0, 0:HD],
                    xs_sb[:, tj // 4, cc, (tj % 4) * P:(tj % 4 + 1) * P],
                    wv_sb[:, cc, :],
                    start=(cc == 0),
                    stop=(cc == CC - 1),
                )
            psv = ps[:, 0, 0:HD].rearrange("p (hp hi d) -> p hp hi d", hi=2, d=D)
            nc.vector.tensor_tensor(
                v_sb[:, tj, :, 0, 0:D], psv[:, :, 0, :], bv_v[:, :, 0, :], ALU.add
            )
            nc.vector.tensor_tensor(
                v_sb[:, tj, :, 1, D:P], psv[:, :, 1, :], bv_v[:, :, 1, :], ALU.add
            )

        # main phase 1: t-quarters 0-2 of qT/kT + v chunks 0-11. The rest
        # (quarter 3 + v 12-15) becomes attention-phase PE filler. co-outer
        # order so the second group (wk co=0, own DMA queue) never waits on
        # the wq co=1 transfer.
        for tsl in range(1):
            for co in range(2):
                for W_s, boff, dest in (
                    (wq_sb, OFF_BQ, qT_sb),
                    (wk_sb, OFF_BK, kT_sb),
                ):
                    emit_qk(
                        W_s, cst_sb[:, boff + co:boff + co + 1], dest, co, tsl
                    )
            for tj in range(4 * tsl, 4 * tsl + 4):
                emit_v(tj)

        # quarters 2-3 of phase 1 are filler inside the attention phase (the
        # PE runs ~640ns/chunk vs ACT's ~1.1us/chunk there); tagged with the
        # t-quarter so blocks that need them can force-drain first
        ph1_fill = []
        for tsl in (1, 2, 3):
            for W_s, boff, dest, kind in (
                (wq_sb, OFF_BQ, qT_sb, "q"),
                (wk_sb, OFF_BK, kT_sb, "k"),
            ):
                for co in range(2):
                    for half in range(2):
                        ph1_fill.append((tsl, kind, lambda
                            W_s=W_s, boff=boff, dest=dest, co=co, tsl=tsl,
                            half=half: emit_qk_half(
                                W_s, cst_sb[:, boff + co:boff + co + 1],
                                dest, co, tsl, half,
                            )))
            for tj in range(4 * tsl, 4 * tsl + 4):
                ph1_fill.append((tsl, "v", lambda tj=tj: emit_v(tj)))
        proj_fill = []

        # ---------------- phase 2: attention + woven proj ----------------
        ot_tiles = {}

        def emit_proj_half(tj, co):
            if co == 0:
                ot_tiles[tj] = ph3.tile([P, C], f16, tag="ot", name=f"ot{tj}")
            ot = ot_tiles[tj]
            pps = ps2s.tile([P, 2, 512], f32, tag="s")
            for dc in range(2):
                nc.tensor.matmul(
                    pps[:, co, :],
                    yT_sb[:, dc, tj * P:(tj + 1) * P],
                    wp_sb[:, dc, co * 512:(co + 1) * 512],
                    start=(dc == 0),
                    stop=(dc == 1),
                )
            # last i-chunk: ACT has no exp left, split the copies so the
            # tail drains twice as fast
            if tj >= 12 and co == 1:
                nc.scalar.copy(ot[:, co * 512:(co + 1) * 512], pps[:, co, :])
            else:
                nc.vector.tensor_copy(
                    ot[:, co * 512:(co + 1) * 512], pps[:, co, :]
                )
            if co == 1:
                nc.sync.dma_start(out[tj * P:(tj + 1) * P, :], ot[:])
                del ot_tiles[tj]

        # emitted-work accounting (ns) to pace fillers: the PE stream should
        # stay at least as long as the ACT (exp) stream it depends on
        clk = {"pe": 0.0, "act": 0.0}

        def fill(n):
            for _ in range(n):
                if ph1_fill:
                    q, kind, fn = ph1_fill.pop(0)
                    fn()
                    clk["pe"] += 853.0
                elif proj_fill:
                    proj_fill.pop(0)()
                    clk["pe"] += 450.0
                else:
                    return

        def fill_to_rate():
            while (ph1_fill or proj_fill) and clk["act"] > clk["pe"]:
                fill(1)

        def drain_ph1(upto_quarter, kinds=("q", "k", "v")):
            i = 0
            while i < len(ph1_fill):
                q, kind, fn = ph1_fill[i]
                if q <= upto_quarter and kind in kinds:
                    ph1_fill.pop(i)
                    fn()
                else:
                    i += 1

        def emit_norm(hp, i0, av0, av1):
            # den rows were already copied into dsb right after the block's
            # last AV; broadcast + reciprocal + scale into yT
            bps = ps2s.tile([P, 2, 512], f32, tag="s")
            nc.tensor.matmul(
                bps[:, 0, :], sel_v, dsb[:],
                start=True, stop=True, skip_group_check=True,
            )
            rec = ph2.tile([P, 512], f32, tag="rec")
            nc.vector.reciprocal_approx_fast(rec[:, :], bps[:, 0, :])
            nc.vector.tensor_tensor(
                yT_sb[0:D, hp, i0:i0 + 512], av0[0:D, :], rec[0:D, :], ALU.mult
            )
            nc.vector.tensor_tensor(
                yT_sb[D:P, hp, i0:i0 + 512], av1[D:P, :], rec[D:P, :], ALU.mult
            )
            if hp == 1 and i0 < 1536:
                ci = i0 // 512
                for tj in range(4 * ci, 4 * ci + 4):
                    for co in range(2):
                        proj_fill.append(
                            lambda tj=tj, co=co: emit_proj_half(tj, co)
                        )

        pending = None
        for ci in range(IC):
            i0 = ci * 512
            njc = 4 * (ci + 1)
            if ci >= 1:
                # only this i-chunk's qT is needed before the block starts;
                # its kT/v stragglers can drain any time before the diagonal
                drain_ph1(ci, kinds=("q",))
            for hp in range(2):
                av0 = ps2a.tile([P, 512], f32, tag="av0")
                av1 = ps2a.tile([P, 512], f32, tag="av1")

                def emit_s(jc):
                    diag = jc >= 4 * ci
                    o = (jc - 4 * ci) if diag else 0
                    c0 = o * P
                    sps = ps2s.tile([P, 2, 512], f32, tag="s")
                    for hi in range(2):
                        bp = D * hi
                        nc.tensor.matmul(
                            sps[:, hi, c0:512],
                            kT_sb[bp:bp + D, hp, jc * P:(jc + 1) * P],
                            qT_sb[bp:bp + D, hp, i0 + c0:i0 + 512],
                            start=True,
                            stop=True,
                            skip_group_check=True,
                        )
                    ex = ph2.tile([P, 2, 512], f16, tag="ex")
                    nc.scalar.activation(
                        ex[:, :, c0:512],
                        sps[:, :, c0:512],
                        ACTF.Exp,
                        scale=float(D) ** -0.5,
                    )
                    if diag:
                        nc.vector.tensor_tensor(
                            ex[:, :, c0:c0 + P],
                            ex[:, :, c0:c0 + P],
                            tri_v[:, None, :].to_broadcast([P, 2, P]),
                            ALU.mult,
                        )
                    return ex, c0

                def emit_av(jc, ex, c0):
                    for hi, av in ((0, av0), (1, av1)):
                        nc.tensor.matmul(
                            av[:, c0:512],
                            v_sb[:, jc, hp, hi, :],
                            ex[:, hi, c0:512],
                            start=(jc == 0),
                            stop=(jc == njc - 1),
                            skip_group_check=True,
                        )

                # score jc+1 issues before AV jc so the in-order PE stream
                # never waits on the ACT exp; fillers pad the PE to ACT rate;
                # the previous block's normalization lands at jc==3, behind a
                # filler, so its broadcast-matmul never waits on the den rows
                pend_av = None
                for jc in range(njc):
                    if ci >= 1 and jc == 4 * ci:
                        drain_ph1(ci)
                    ex, c0 = emit_s(jc)
                    w = 512 - (jc - 4 * ci) * P if jc >= 4 * ci else 512
                    clk["pe"] += w / 2.4 + 8
                    clk["act"] += 2 * w / 1.2 + 240
                    if jc >= 4 * ci:
                        clk["act"] += 300
                    # fillers go between the score and the AV that depends on
                    # the previous chunk's exp, absorbing ACT-rate jitter
                    if (ci, hp) != (0, 0) or jc >= 3:
                        fill_to_rate()
                    if pend_av is not None:
                        emit_av(*pend_av)
                        clk["pe"] += 2 * (512 - pend_av[2]) / 2.4 + 16
                    pend_av = (jc, ex, c0)
                    if jc == 3 and pending is not None:
                        emit_norm(*pending)
                        pending = None
                        clk["pe"] += 230.0
                emit_av(*pend_av)
                clk["pe"] += 2 * (512 - pend_av[2]) / 2.4 + 16
                # den rows -> staging now; the rest of the normalization is
                # emitted inside the next block so the PE never stalls on it
                nc.vector.tensor_copy(dsb[D:D + 1, :], av0[D:D + 1, :])
                nc.vector.tensor_copy(dsb[0:1, :], av1[0:1, :])
                pending = (hp, i0, av0, av1)
        # final block: normalize in 128-column slices, each immediately
        # followed by the projection tile it unblocks, so the tail pipelines
        # instead of serializing norm -> all projs -> DMA
        fill(1)
        hp, i0, av0, av1 = pending
        bps = ps2s.tile([P, 2, 512], f32, tag="s")
        nc.tensor.matmul(
            bps[:, 0, :], sel_v, dsb[:],
            start=True, stop=True, skip_group_check=True,
        )
        rec = ph2.tile([P, 512], f32, tag="rec")
        nc.vector.reciprocal_approx_fast(rec[:, :], bps[:, 0, :])
        for sl in range(4):
            s0, s1 = sl * P, (sl + 1) * P
            nc.vector.tensor_tensor(
                yT_sb[0:D, hp, i0 + s0:i0 + s1], av0[0:D, s0:s1],
                rec[0:D, s0:s1], ALU.mult,
            )
            nc.vector.tensor_tensor(
                yT_sb[D:P, hp, i0 + s0:i0 + s1], av1[D:P, s0:s1],
                rec[D:P, s0:s1], ALU.mult,
            )
            for co in range(2):
                emit_proj_half(12 + sl, co)
        while proj_fill:
            proj_fill.pop(0)()
    nc.compile()
    return nc


def _get_nc():
    global _NC
    if _NC is None:
        _NC = _build_nc()
    return _NC


def _pack_inputs(x_b, W_qkv, b_qkv, W_proj, g):
    """Host-side packing for core (batch, head-group g): fp16, DMA-friendly."""
    f16 = np.float16
    s0 = HD * g
    xt = np.ascontiguousarray(x_b.T).astype(f16)          # [C, T]
    xqa = np.ascontiguousarray(
        xt.reshape(CC, P, 4, 512).transpose(1, 2, 0, 3)   # [p, quarter, o, t]
    )

    def wpack(col0):
        w = W_qkv[:, col0:col0 + HD].astype(f16)          # [C, HD]
        return np.ascontiguousarray(w.reshape(CC, P, 2, P).transpose(1, 2, 0, 3))

    wv_ = W_qkv[:, 2 * C + s0:2 * C + s0 + HD].astype(f16)
    wv_p = np.ascontiguousarray(wv_.reshape(CC, P, HD).transpose(1, 0, 2))
    wp_ = W_proj[s0:s0 + HD, :].astype(f16)               # [HD, C]
    wp_p = np.ascontiguousarray(wp_.reshape(2, P, C).transpose(1, 0, 2))

    cstm = np.zeros((P, CSTW), dtype=f16)
    cstm[:, OFF_TRI:OFF_TRI + P] = np.triu(np.ones((P, P), dtype=f16))
    cstm[D, OFF_SEL:OFF_SEL + D] = 1.0
    cstm[0, OFF_SEL + D:OFF_SEL + P] = 1.0
    cstm[:, OFF_BQ:OFF_BQ + 2] = b_qkv[s0:s0 + HD].reshape(2, P).T
    cstm[:, OFF_BK:OFF_BK + 2] = (
        b_qkv[C + s0:C + s0 + HD].reshape(2, P).T
    )
    cstm[:, OFF_BV:OFF_BV + HD] = b_qkv[2 * C + s0:2 * C + s0 + HD]

    return {
        "xq": xqa,
        "wq2": wpack(s0),
        "wk2": wpack(C + s0),
        "wv2": wv_p,
        "wp2": wp_p,
        "cst": np.ascontiguousarray(cstm),
    }


def kernel(x, W_qkv, b_qkv, W_proj, b_proj):
    global LAST_RESULTS
    from concourse import bass_utils

    x = np.asarray(x, dtype=np.float32)
    W_qkv = np.asarray(W_qkv, dtype=np.float32)
    b_qkv = np.asarray(b_qkv, dtype=np.float32)
    W_proj = np.asarray(W_proj, dtype=np.float32)
    b_proj = np.asarray(b_proj, dtype=np.float32)

    nc = _get_nc()
    in_maps = []
    for c in range(8):
        b, g = divmod(c, 4)
        in_maps.append(_pack_inputs(x[b], W_qkv, b_qkv, W_proj, g))

    res = bass_utils.run_bass_kernel_spmd(nc, in_maps, core_ids=list(range(8)))
    LAST_RESULTS = res
    ys = []
    for b in range(2):
        y = res.results[4 * b]["out"].astype(np.float64)
        for g in range(1, 4):
            y = y + res.results[4 * b + g]["out"]
        ys.append((y + b_proj).astype(np.float32))
    return np.stack(ys, axis=0)
